# revision 13
# baseline (speedup 1.0000x reference)
"""Distributed BatchSpectralLoss kernel for Trainium2 (8 NeuronCores).

Computes sum of top-k squared singular values of x (= top-k eigenvalues of
the Gram matrix G = x^T x) for x of shape (8192, 4096), k small (k=1).

Algorithm (all device matmuls in bf16 with fp32 PSUM accumulation):
  1. Host: estimate lambda_1 cheaply (block power iteration), scale x by
     1/sqrt(C) and cast to bf16, so the device Gram directly yields A0 = G/C.
  2. Device, sharded across 8 cores (core r owns rows r*512..(r+1)*512 of
     every produced matrix; AllGather of row slices re-assembles plain
     row-major full matrices):
       - g_slice = xcols_r^T @ x          (A0 row-slice)        + AllGather
       - p1 squarings: a_slice = a_slice^T(kxm) @ a_full        + AllGather
         (A_{j+1} = A_j^2; symmetry lets the core's own row-slice, read
          transposed, serve as the lhsT column-slice)
       - block power phase (hand-written, persistent SBUF, `chains`
         independent chains interleaved so one chain's matmuls hide the
         other's AllGather): y_slice = A_p1[rows_r,:] @ y_full, AllGather
       - w = one more application per chain (no AllGather)
       - S1[ci,cj] = Y_ci^T W_cj, S0[ci,cj] = Y_ci^T Y_cj  (partial over the
         core's rows; host sums partials over cores)
  3. Host: generalized Ritz eigenvalues theta_i of (S1, S0) approximate
     lambda_i(A_p1); lambda_i(G) = C * theta_i^(1/2^p1); answer = sum top k.

The 2^-p1 root compresses block-phase and late rounding errors by 2^p1; the
p1 squarings make s block steps act like a degree s*2^p1 polynomial filter.
"""

import numpy as np
import ml_dtypes

N_CORES = 8
M_ROWS = 8192
N_DIM = 4096
P1 = 2
S_STEPS = 10
B_BLOCK = 128
CHAINS = 2

_NC_CACHE: dict = {}


def _est_scale(x_np, iters=15, blk=4):
    """Host block-power-iteration estimate of lambda_1(x^T x).

    Only used to pick the static normalization C; range safety needs C within
    ~±20% of lambda_1, which ~15 block iterations comfortably deliver for any
    PSD spectrum. Returns 1.10 * max Rayleigh quotient (mild overshoot keeps
    the squaring chain's magnitudes shrinking rather than growing).
    """
    rng = np.random.default_rng(0)
    v = rng.standard_normal((x_np.shape[1], blk)).astype(np.float32)
    v /= np.linalg.norm(v, axis=0, keepdims=True)
    for _ in range(iters):
        w = x_np.T @ (x_np @ v)
        v = w / np.linalg.norm(w, axis=0, keepdims=True)
    x64 = x_np.astype(np.float64)
    v64 = v.astype(np.float64)
    v64 /= np.linalg.norm(v64, axis=0, keepdims=True)
    ray = ((x64 @ v64) ** 2).sum(axis=0)
    return 1.10 * float(ray.max())


def _build_nc(m_rows, n_dim, b, p1, s, n_cores, chains, enable_asserts=False):
    import concourse.mybir as mybir
    import concourse.tile as tile
    from concourse import bacc
    import concourse.kernels.tile_matmul as tm
    from contextlib import ExitStack

    orig_comp = tm.composable_matmul_tile_kernel

    def comp_psum2(*a, **kw):
        kw.setdefault("psum_n_bufs", 2)
        return orig_comp(*a, **kw)

    def matmul_tile_kernel(*a, **kw):
        tm.composable_matmul_tile_kernel = comp_psum2
        try:
            return tm.matmul_tile_kernel(*a, **kw)
        finally:
            tm.composable_matmul_tile_kernel = orig_comp

    P = 128
    sl = n_dim // n_cores  # 512 rows per core
    msub = sl // P         # 4
    kpo = n_dim // P       # 32
    bf = mybir.dt.bfloat16
    f32 = mybir.dt.float32
    nc = bacc.Bacc(
        "TRN2",
        target_bir_lowering=False,
        debug=False,
        enable_asserts=enable_asserts,
        num_devices=n_cores,
    )

    xb = nc.dram_tensor("xb", [m_rows, n_dim], bf, kind="ExternalInput")
    xcols = nc.dram_tensor("xcols", [m_rows, sl], bf, kind="ExternalInput")
    omegas = [
        nc.dram_tensor(f"omega{c}", [n_dim, b], bf, kind="ExternalInput")
        for c in range(chains)
    ]
    nb = chains * b
    s1r = nc.dram_tensor("s1r", [nb, nb], f32, kind="ExternalOutput")
    s0r = nc.dram_tensor("s0r", [nb, nb], f32, kind="ExternalOutput")

    y_slice = [
        [nc.dram_tensor(f"y_slice_{c}_{t}", [sl, b], bf) for t in range(s)]
        for c in range(chains)
    ]
    y_full = [
        [
            nc.dram_tensor(f"y_full_{c}_{t}", [n_dim, b], bf, addr_space="Shared")
            for t in range(s)
        ]
        for c in range(chains)
    ]

    rg = [list(range(n_cores))]

    def dve_evict(nc_, psum, sbuf):
        nc_.vector.tensor_copy(out=sbuf, in_=psum)

    def ag(inp, outp):
        nc.gpsimd.collective_compute(
            "AllGather",
            mybir.AluOpType.bypass,
            replica_groups=rg,
            ins=[inp.ap().opt()],
            outs=[outp.ap().opt()],
        )

    from concourse.kernels.tile_matmul import (
        batched_producer_kxm,
        batched_producer_kxn,
        composable_matmul_tile_kernel,
        dma_from_dram_kxm,
        dma_from_dram_kxn,
        dma_to_dram_mxn,
    )

    def dve_reducer(nc_, psum, sbuf, md):
        nc_.vector.tensor_copy(out=sbuf, in_=psum)

    def sq_half(tc, kxm_pool, kxn_pool, srcL, srcR, kxn_full, out_t):
        """out_t[sl, half] = A[rows_r,:] @ A[:, half-cols], with A's row-slice
        stored as column-half tensors srcL/srcR (read transposed as k-batched
        kxm) and the gathered half kxn_full [n_dim, half] k-split by rows."""
        pL, sL = dma_from_dram_kxm(kxm_pool, srcL.ap(), transpose_ap=True)
        pR, sR = dma_from_dram_kxm(kxm_pool, srcR.ap(), transpose_ap=True)
        kxm_producer, kxm_shape = batched_producer_kxm(
            [pL, pR], [sL, sR], batch_dim="k"
        )
        nrows = kxn_full.shape[0]
        qL, tL = dma_from_dram_kxn(kxn_pool, kxn_full.ap()[0 : nrows // 2, :])
        qR, tR = dma_from_dram_kxn(kxn_pool, kxn_full.ap()[nrows // 2 :, :])
        kxn_producer, kxn_shape = batched_producer_kxn(
            [qL, qR], [tL, tR], batch_dim="k"
        )
        composable_matmul_tile_kernel(
            tc=tc,
            kxm_shape=kxm_shape,
            kxn_shape=kxn_shape,
            output_type=bf,
            kxm_producer=kxm_producer,
            kxn_producer=kxn_producer,
            mxn_consumer=dma_to_dram_mxn(out_t.ap()),
            mxn_subtile_reducer=dve_reducer,
            psum_n_bufs=2,
        )

    half = n_dim // 2
    gL = nc.dram_tensor("gL", [sl, half], bf)
    gR = nc.dram_tensor("gR", [sl, half], bf)
    aL0 = nc.dram_tensor("aL0", [n_dim, half], bf, addr_space="Shared")
    aR0 = nc.dram_tensor("aR0", [n_dim, half], bf, addr_space="Shared")
    s1L = nc.dram_tensor("s1L", [sl, half], bf)
    s1R = nc.dram_tensor("s1R", [sl, half], bf)
    aL1 = nc.dram_tensor("aL1", [n_dim, half], bf, addr_space="Shared")
    aR1 = nc.dram_tensor("aR1", [n_dim, half], bf, addr_space="Shared")
    s2L = nc.dram_tensor("s2L", [sl, half], bf)
    s2R = nc.dram_tensor("s2R", [sl, half], bf)

    with tile.TileContext(nc) as tc:
        # ---- Gram row-slice in column halves; AG each half when ready ----
        matmul_tile_kernel(
            tc, kxm_ap=xcols.ap(), kxn_ap=xb.ap()[:, 0:half], mxn_ap=gL.ap(),
            psum_evict_fn=dve_evict,
        )
        ag(gL, aL0)
        matmul_tile_kernel(
            tc, kxm_ap=xcols.ap(), kxn_ap=xb.ap()[:, half:], mxn_ap=gR.ap(),
            psum_evict_fn=dve_evict,
        )
        ag(gR, aR0)

        # ---- squaring 1 (halves, AGs overlap the other half's matmuls) ----
        from contextlib import ExitStack as _ES

        with _ES() as sqctx:
            kxm_pool = sqctx.enter_context(tc.tile_pool(name="sq_kxm", bufs=9))
            kxn_pool = sqctx.enter_context(tc.tile_pool(name="sq_kxn", bufs=3))
            sq_half(tc, kxm_pool, kxn_pool, gL, gR, aL0, s1L)
            ag(s1L, aL1)
            sq_half(tc, kxm_pool, kxn_pool, gL, gR, aR0, s1R)
            ag(s1R, aR1)
            # ---- squaring 2 (no AG after; block uses local slices) ----
            sq_half(tc, kxm_pool, kxn_pool, s1L, s1R, aL1, s2L)
            sq_half(tc, kxm_pool, kxn_pool, s1L, s1R, aR1, s2R)

        # ---- hand-written block power phase ----
        with ExitStack() as ctx:
            cpool = ctx.enter_context(tc.tile_pool(name="blk_const", bufs=1))
            ypool = ctx.enter_context(tc.tile_pool(name="blk_y", bufs=3))
            opool = ctx.enter_context(tc.tile_pool(name="blk_out", bufs=3))
            # PSUM budget: 8 banks total = chains*3 (block) + 2 (S-forms)
            pspool = ctx.enter_context(
                tc.tile_pool(name="blk_psum", bufs=3, space="PSUM")
            )

            # kc[pi, po, f] = A_p1[row f (slice-local), col po*128+pi]
            kc = cpool.tile([P, kpo, sl], bf, tag="kc")
            for src_t, off in ((s2L, 0), (s2R, kpo // 2)):
                s3 = src_t.ap().rearrange("f (po pi) -> f po pi", pi=P)
                for kt in range(kpo // 8):
                    nc.sync.dma_start_transpose(
                        kc[:, off + kt * 4 : off + (kt + 1) * 4, :],
                        s3[:, kt * 4 : (kt + 1) * 4, :],
                    )

            y_cur = []
            for c in range(chains):
                ysb = ypool.tile([P, kpo, b], bf, tag=f"yf{c}")
                nc.sync.dma_start(
                    ysb[:], omegas[c].ap().rearrange("(po pi) b -> pi po b", pi=P)
                )
                y_cur.append(ysb)

            def chain_step(c, dst_sb):
                """dst_sb[pi, mo, :] = (A_p1 @ Y_c)[rows_r] in bf16."""
                for mo in range(msub):
                    ps = pspool.tile([P, b], f32, tag=f"ps{c}")
                    for ko in range(kpo):
                        nc.tensor.matmul(
                            ps[:],
                            kc[:, ko, mo * P : (mo + 1) * P],
                            y_cur[c][:, ko, :],
                            start=(ko == 0),
                            stop=(ko == kpo - 1),
                        )
                    nc.vector.tensor_copy(dst_sb[:, mo, :], ps[:])

            ys_final = [None] * chains
            for t in range(s):
                for c in range(chains):
                    last = t == s - 1
                    if last:
                        out_sb = cpool.tile([P, msub, b], bf, tag=f"ysfin{c}")
                        ys_final[c] = out_sb
                    else:
                        out_sb = opool.tile([P, msub, b], bf, tag=f"yo{c}")
                    chain_step(c, out_sb)
                    nc.sync.dma_start(
                        y_slice[c][t].ap().rearrange("(mo pi) b -> pi mo b", pi=P),
                        out_sb[:],
                    )
                    ag(y_slice[c][t], y_full[c][t])
                    ysb = ypool.tile([P, kpo, b], bf, tag=f"yf{c}")
                    nc.sync.dma_start(
                        ysb[:],
                        y_full[c][t].ap().rearrange("(po pi) b -> pi po b", pi=P),
                    )
                    y_cur[c] = ysb

            # one more application per chain: W_c = A_p1 @ Y_c (slice, no AG)
            w_sb = []
            for c in range(chains):
                wsb = cpool.tile([P, msub, b], bf, tag=f"w{c}")
                chain_step(c, wsb)
                w_sb.append(wsb)

            # ---- S forms: S1[ci,cj] = Y_ci^T W_cj, S0[ci,cj] = Y_ci^T Y_cj ----
            spool = ctx.enter_context(tc.tile_pool(name="s_out", bufs=2))
            pspool2 = ctx.enter_context(
                tc.tile_pool(name="s_psum", bufs=2, space="PSUM")
            )
            for ci in range(chains):
                for cj in range(chains):
                    for which, rhs_sb, out_t in (
                        ("s1", w_sb[cj], s1r),
                        ("s0", ys_final[cj], s0r),
                    ):
                        ps = pspool2.tile([b, b], f32, tag="ps_s")
                        for ko in range(msub):
                            nc.tensor.matmul(
                                ps[:],
                                ys_final[ci][:, ko, :],
                                rhs_sb[:, ko, :],
                                start=(ko == 0),
                                stop=(ko == msub - 1),
                            )
                        osb = spool.tile([b, b], f32, tag="osb")
                        nc.vector.tensor_copy(osb[:], ps[:])
                        nc.sync.dma_start(
                            out_t.ap()[ci * b : (ci + 1) * b, cj * b : (cj + 1) * b],
                            osb[:],
                        )

    nc.compile()
    return nc


def _get_nc(cfg):
    if cfg not in _NC_CACHE:
        _NC_CACHE[cfg] = _build_nc(*cfg)
    return _NC_CACHE[cfg]


def _ritz_topk(S1, S0, k):
    """Top-k generalized eigenvalues of (S1, S0), f64, rank-guarded."""
    S1 = 0.5 * (S1 + S1.T)
    S0 = 0.5 * (S0 + S0.T)
    d = np.sqrt(np.clip(np.diag(S0), 0, None))
    d = np.where(d > 0, d, 1.0)
    dn = 1.0 / d
    S0n = S0 * dn[:, None] * dn[None, :]
    S1n = S1 * dn[:, None] * dn[None, :]
    w0, v0 = np.linalg.eigh(S0n)
    keep = w0 > (w0.max() * 1e-4)
    v = v0[:, keep] / np.sqrt(w0[keep])[None, :]
    m = v.T @ S1n @ v
    m = 0.5 * (m + m.T)
    ev = np.linalg.eigvalsh(m)
    ev = np.clip(ev, 0.0, None)
    return np.sort(ev)[::-1][:k]


def _host_solve(results, k, p1, c_scale):
    S1 = np.zeros_like(results[0]["s1r"], dtype=np.float64)
    S0 = np.zeros_like(results[0]["s0r"], dtype=np.float64)
    for r in results:
        S1 += r["s1r"].astype(np.float64)
        S0 += r["s0r"].astype(np.float64)
    thetas = _ritz_topk(S1, S0, k)
    lams = c_scale * np.power(np.clip(thetas, 1e-300, None), 1.0 / (1 << p1))
    return float(np.sum(lams))


def _make_inputs(x_np, b, n_cores, c_scale, chains):
    n_dim = x_np.shape[1]
    sl = n_dim // n_cores
    bf = ml_dtypes.bfloat16
    xs = (x_np.astype(np.float64) / np.sqrt(c_scale)).astype(np.float32)
    xb = np.ascontiguousarray(xs.astype(bf))
    rng = np.random.default_rng(12345)
    omegas = [
        np.ascontiguousarray(
            rng.standard_normal((n_dim, b)).astype(np.float32).astype(bf)
        )
        for _ in range(chains)
    ]
    in_maps = []
    for r in range(n_cores):
        m = {
            "xb": xb,
            "xcols": np.ascontiguousarray(xb[:, r * sl : (r + 1) * sl]),
        }
        for c in range(chains):
            m[f"omega{c}"] = omegas[c]
        in_maps.append(m)
    return in_maps


def _host_fallback(x_np, k_int):
    """Correct-but-slow host path, used only if the device result is bad."""
    import scipy.linalg

    g = x_np.astype(np.float64).T @ x_np.astype(np.float64)
    n = g.shape[0]
    ev = scipy.linalg.eigh(g, eigvals_only=True, subset_by_index=[n - k_int, n - 1])
    return float(np.sum(ev))


def kernel(x, k):
    from concourse.bass_utils import run_bass_kernel_spmd

    x_np = np.asarray(x, dtype=np.float32)
    k_int = int(np.asarray(k))
    if k_int <= 0:
        return np.asarray(0.0, dtype=np.float32)

    try:
        c_scale = _est_scale(x_np)
        cfg = (M_ROWS, N_DIM, B_BLOCK, P1, S_STEPS, N_CORES, CHAINS)
        nc = _get_nc(cfg)
        in_maps = _make_inputs(x_np, B_BLOCK, N_CORES, c_scale, CHAINS)
        res = run_bass_kernel_spmd(nc, in_maps, core_ids=list(range(N_CORES)))
        val = _host_solve(res.results, k_int, P1, c_scale)
        if not np.isfinite(val) or val <= 0:
            raise FloatingPointError(f"bad device result {val}")
    except Exception:
        val = _host_fallback(x_np, k_int)
    return np.asarray(val, dtype=np.float32)


# revision 14
# speedup vs baseline: 1.0280x; 1.0280x over previous
"""Distributed BatchSpectralLoss kernel for Trainium2 (8 NeuronCores).

Computes sum of top-k squared singular values of x (= top-k eigenvalues of
the Gram matrix G = x^T x) for x of shape (8192, 4096), k small (k=1).

Algorithm (all device matmuls in bf16 with fp32 PSUM accumulation):
  1. Host: estimate lambda_1 cheaply (block power iteration), scale x by
     1/sqrt(C) and cast to bf16, so the device Gram directly yields A0 = G/C.
  2. Device, sharded across 8 cores (core r owns rows r*512..(r+1)*512 of
     every produced matrix; AllGather of row slices re-assembles plain
     row-major full matrices):
       - g_slice = xcols_r^T @ x          (A0 row-slice)        + AllGather
       - p1 squarings: a_slice = a_slice^T(kxm) @ a_full        + AllGather
         (A_{j+1} = A_j^2; symmetry lets the core's own row-slice, read
          transposed, serve as the lhsT column-slice)
       - block power phase (hand-written, persistent SBUF, `chains`
         independent chains interleaved so one chain's matmuls hide the
         other's AllGather): y_slice = A_p1[rows_r,:] @ y_full, AllGather
       - w = one more application per chain (no AllGather)
       - S1[ci,cj] = Y_ci^T W_cj, S0[ci,cj] = Y_ci^T Y_cj  (partial over the
         core's rows; host sums partials over cores)
  3. Host: generalized Ritz eigenvalues theta_i of (S1, S0) approximate
     lambda_i(A_p1); lambda_i(G) = C * theta_i^(1/2^p1); answer = sum top k.

The 2^-p1 root compresses block-phase and late rounding errors by 2^p1; the
p1 squarings make s block steps act like a degree s*2^p1 polynomial filter.
"""

import numpy as np
import ml_dtypes

N_CORES = 8
M_ROWS = 8192
N_DIM = 4096
P1 = 2
S_STEPS = 10
B_BLOCK = 128
CHAINS = 2

_NC_CACHE: dict = {}


def _est_scale(x_np, iters=15, blk=4):
    """Host block-power-iteration estimate of lambda_1(x^T x).

    Only used to pick the static normalization C; range safety needs C within
    ~±20% of lambda_1, which ~15 block iterations comfortably deliver for any
    PSD spectrum. Returns 1.10 * max Rayleigh quotient (mild overshoot keeps
    the squaring chain's magnitudes shrinking rather than growing).
    """
    rng = np.random.default_rng(0)
    v = rng.standard_normal((x_np.shape[1], blk)).astype(np.float32)
    v /= np.linalg.norm(v, axis=0, keepdims=True)
    for _ in range(iters):
        w = x_np.T @ (x_np @ v)
        v = w / np.linalg.norm(w, axis=0, keepdims=True)
    x64 = x_np.astype(np.float64)
    v64 = v.astype(np.float64)
    v64 /= np.linalg.norm(v64, axis=0, keepdims=True)
    ray = ((x64 @ v64) ** 2).sum(axis=0)
    return 1.10 * float(ray.max())


def _build_nc(m_rows, n_dim, b, p1, s, n_cores, chains, enable_asserts=False):
    import concourse.mybir as mybir
    import concourse.tile as tile
    from concourse import bacc
    import concourse.kernels.tile_matmul as tm
    from contextlib import ExitStack

    orig_comp = tm.composable_matmul_tile_kernel

    def comp_psum2(*a, **kw):
        kw.setdefault("psum_n_bufs", 2)
        return orig_comp(*a, **kw)

    def matmul_tile_kernel(*a, **kw):
        tm.composable_matmul_tile_kernel = comp_psum2
        try:
            return tm.matmul_tile_kernel(*a, **kw)
        finally:
            tm.composable_matmul_tile_kernel = orig_comp

    P = 128
    sl = n_dim // n_cores  # 512 rows per core
    msub = sl // P         # 4
    kpo = n_dim // P       # 32
    bf = mybir.dt.bfloat16
    f32 = mybir.dt.float32
    nc = bacc.Bacc(
        "TRN2",
        target_bir_lowering=False,
        debug=False,
        enable_asserts=enable_asserts,
        num_devices=n_cores,
    )

    xb = nc.dram_tensor("xb", [m_rows, n_dim], bf, kind="ExternalInput")
    xcols = nc.dram_tensor("xcols", [m_rows, sl], bf, kind="ExternalInput")
    omegas = [
        nc.dram_tensor(f"omega{c}", [n_dim, b], bf, kind="ExternalInput")
        for c in range(chains)
    ]
    nb = chains * b
    s1r = nc.dram_tensor("s1r", [nb, nb], f32, kind="ExternalOutput")
    s0r = nc.dram_tensor("s0r", [nb, nb], f32, kind="ExternalOutput")

    g_slice = nc.dram_tensor("g_slice", [sl, n_dim], bf)
    a_full = [
        nc.dram_tensor(f"a_full_{j}", [n_dim, n_dim], bf, addr_space="Shared")
        for j in range(p1)
    ]
    a_slice = [nc.dram_tensor(f"a_slice_{j}", [sl, n_dim], bf) for j in range(p1)]
    y_slice = [
        [nc.dram_tensor(f"y_slice_{c}_{t}", [sl, b], bf) for t in range(s)]
        for c in range(chains)
    ]
    y_full = [
        [
            nc.dram_tensor(f"y_full_{c}_{t}", [n_dim, b], bf, addr_space="Shared")
            for t in range(s)
        ]
        for c in range(chains)
    ]

    rg = [list(range(n_cores))]

    def dve_evict(nc_, psum, sbuf):
        nc_.vector.tensor_copy(out=sbuf, in_=psum)

    def ag(inp, outp):
        nc.gpsimd.collective_compute(
            "AllGather",
            mybir.AluOpType.bypass,
            replica_groups=rg,
            ins=[inp.ap().opt()],
            outs=[outp.ap().opt()],
        )

    with tile.TileContext(nc) as tc:
        # ---- Gram row-slice: G[rows_r, :] = xcols^T @ x ----
        matmul_tile_kernel(
            tc, kxm_ap=xcols.ap(), kxn_ap=xb.ap(), mxn_ap=g_slice.ap(),
            psum_evict_fn=dve_evict,
        )
        prev_s = g_slice
        if p1 > 0:
            ag(g_slice, a_full[0])
            prev_f = a_full[0]
            for j in range(p1):
                matmul_tile_kernel(
                    tc,
                    kxm_ap=prev_s.ap(),
                    kxn_ap=prev_f.ap(),
                    mxn_ap=a_slice[j].ap(),
                    transpose_kxm=True,
                    psum_evict_fn=dve_evict,
                )
                prev_s = a_slice[j]
                if j + 1 < p1:
                    ag(prev_s, a_full[j + 1])
                    prev_f = a_full[j + 1]

        ak = prev_s  # [sl, n_dim] row-slice of A_{p1} (= its column-slice^T)

        # ---- hand-written block power phase ----
        with ExitStack() as ctx:
            cpool = ctx.enter_context(tc.tile_pool(name="blk_const", bufs=1))
            ypool = ctx.enter_context(tc.tile_pool(name="blk_y", bufs=3))
            opool = ctx.enter_context(tc.tile_pool(name="blk_out", bufs=3))
            # PSUM budget: 8 banks total = chains*3 (block) + 2 (S-forms)
            pspool = ctx.enter_context(
                tc.tile_pool(name="blk_psum", bufs=3, space="PSUM")
            )

            # kc[pi, po, f] = A_p1[row f (slice-local), col po*128+pi]
            kc = cpool.tile([P, kpo, sl], bf, tag="kc")
            ak3 = ak.ap().rearrange("f (po pi) -> f po pi", pi=P)
            for kt in range(kpo // 4):
                nc.sync.dma_start_transpose(
                    kc[:, kt * 4 : (kt + 1) * 4, :], ak3[:, kt * 4 : (kt + 1) * 4, :]
                )

            y_cur = []
            for c in range(chains):
                ysb = ypool.tile([P, kpo, b], bf, tag=f"yf{c}")
                nc.sync.dma_start(
                    ysb[:], omegas[c].ap().rearrange("(po pi) b -> pi po b", pi=P)
                )
                y_cur.append(ysb)

            def chain_step(c, dst_sb):
                """dst_sb[pi, mo, :] = (A_p1 @ Y_c)[rows_r] in bf16."""
                for mo in range(msub):
                    ps = pspool.tile([P, b], f32, tag=f"ps{c}")
                    for ko in range(kpo):
                        nc.tensor.matmul(
                            ps[:],
                            kc[:, ko, mo * P : (mo + 1) * P],
                            y_cur[c][:, ko, :],
                            start=(ko == 0),
                            stop=(ko == kpo - 1),
                        )
                    nc.vector.tensor_copy(dst_sb[:, mo, :], ps[:])

            ys_final = [None] * chains
            for t in range(s):
                for c in range(chains):
                    last = t == s - 1
                    if last:
                        out_sb = cpool.tile([P, msub, b], bf, tag=f"ysfin{c}")
                        ys_final[c] = out_sb
                    else:
                        out_sb = opool.tile([P, msub, b], bf, tag=f"yo{c}")
                    chain_step(c, out_sb)
                    nc.sync.dma_start(
                        y_slice[c][t].ap().rearrange("(mo pi) b -> pi mo b", pi=P),
                        out_sb[:],
                    )
                    ag(y_slice[c][t], y_full[c][t])
                    ysb = ypool.tile([P, kpo, b], bf, tag=f"yf{c}")
                    nc.sync.dma_start(
                        ysb[:],
                        y_full[c][t].ap().rearrange("(po pi) b -> pi po b", pi=P),
                    )
                    y_cur[c] = ysb

            # one more application per chain: W_c = A_p1 @ Y_c (slice, no AG)
            w_sb = []
            for c in range(chains):
                wsb = cpool.tile([P, msub, b], bf, tag=f"w{c}")
                chain_step(c, wsb)
                w_sb.append(wsb)

            # ---- S forms: S1[ci,cj] = Y_ci^T W_cj, S0[ci,cj] = Y_ci^T Y_cj ----
            spool = ctx.enter_context(tc.tile_pool(name="s_out", bufs=2))
            pspool2 = ctx.enter_context(
                tc.tile_pool(name="s_psum", bufs=2, space="PSUM")
            )
            for ci in range(chains):
                for cj in range(chains):
                    for which, rhs_sb, out_t in (
                        ("s1", w_sb[cj], s1r),
                        ("s0", ys_final[cj], s0r),
                    ):
                        ps = pspool2.tile([b, b], f32, tag="ps_s")
                        for ko in range(msub):
                            nc.tensor.matmul(
                                ps[:],
                                ys_final[ci][:, ko, :],
                                rhs_sb[:, ko, :],
                                start=(ko == 0),
                                stop=(ko == msub - 1),
                            )
                        osb = spool.tile([b, b], f32, tag="osb")
                        nc.vector.tensor_copy(osb[:], ps[:])
                        nc.sync.dma_start(
                            out_t.ap()[ci * b : (ci + 1) * b, cj * b : (cj + 1) * b],
                            osb[:],
                        )

    nc.compile()
    return nc


def _get_nc(cfg):
    if cfg not in _NC_CACHE:
        _NC_CACHE[cfg] = _build_nc(*cfg)
    return _NC_CACHE[cfg]


def _ritz_topk(S1, S0, k):
    """Top-k generalized eigenvalues of (S1, S0), f64, rank-guarded."""
    S1 = 0.5 * (S1 + S1.T)
    S0 = 0.5 * (S0 + S0.T)
    d = np.sqrt(np.clip(np.diag(S0), 0, None))
    d = np.where(d > 0, d, 1.0)
    dn = 1.0 / d
    S0n = S0 * dn[:, None] * dn[None, :]
    S1n = S1 * dn[:, None] * dn[None, :]
    w0, v0 = np.linalg.eigh(S0n)
    keep = w0 > (w0.max() * 1e-4)
    v = v0[:, keep] / np.sqrt(w0[keep])[None, :]
    m = v.T @ S1n @ v
    m = 0.5 * (m + m.T)
    ev = np.linalg.eigvalsh(m)
    ev = np.clip(ev, 0.0, None)
    return np.sort(ev)[::-1][:k]


def _host_solve(results, k, p1, c_scale):
    S1 = np.zeros_like(results[0]["s1r"], dtype=np.float64)
    S0 = np.zeros_like(results[0]["s0r"], dtype=np.float64)
    for r in results:
        S1 += r["s1r"].astype(np.float64)
        S0 += r["s0r"].astype(np.float64)
    thetas = _ritz_topk(S1, S0, k)
    lams = c_scale * np.power(np.clip(thetas, 1e-300, None), 1.0 / (1 << p1))
    return float(np.sum(lams))


def _make_inputs(x_np, b, n_cores, c_scale, chains):
    n_dim = x_np.shape[1]
    sl = n_dim // n_cores
    bf = ml_dtypes.bfloat16
    xs = (x_np.astype(np.float64) / np.sqrt(c_scale)).astype(np.float32)
    xb = np.ascontiguousarray(xs.astype(bf))
    rng = np.random.default_rng(12345)
    omegas = [
        np.ascontiguousarray(
            rng.standard_normal((n_dim, b)).astype(np.float32).astype(bf)
        )
        for _ in range(chains)
    ]
    in_maps = []
    for r in range(n_cores):
        m = {
            "xb": xb,
            "xcols": np.ascontiguousarray(xb[:, r * sl : (r + 1) * sl]),
        }
        for c in range(chains):
            m[f"omega{c}"] = omegas[c]
        in_maps.append(m)
    return in_maps


def _host_fallback(x_np, k_int):
    """Correct-but-slow host path, used only if the device result is bad."""
    import scipy.linalg

    g = x_np.astype(np.float64).T @ x_np.astype(np.float64)
    n = g.shape[0]
    ev = scipy.linalg.eigh(g, eigvals_only=True, subset_by_index=[n - k_int, n - 1])
    return float(np.sum(ev))


def kernel(x, k):
    from concourse.bass_utils import run_bass_kernel_spmd

    x_np = np.asarray(x, dtype=np.float32)
    k_int = int(np.asarray(k))
    if k_int <= 0:
        return np.asarray(0.0, dtype=np.float32)

    try:
        c_scale = _est_scale(x_np)
        cfg = (M_ROWS, N_DIM, B_BLOCK, P1, S_STEPS, N_CORES, CHAINS)
        nc = _get_nc(cfg)
        in_maps = _make_inputs(x_np, B_BLOCK, N_CORES, c_scale, CHAINS)
        res = run_bass_kernel_spmd(nc, in_maps, core_ids=list(range(N_CORES)))
        val = _host_solve(res.results, k_int, P1, c_scale)
        if not np.isfinite(val) or val <= 0:
            raise FloatingPointError(f"bad device result {val}")
    except Exception:
        val = _host_fallback(x_np, k_int)
    return np.asarray(val, dtype=np.float32)


# revision 15
# speedup vs baseline: 1.0575x; 1.0287x over previous
"""Distributed BatchSpectralLoss kernel for Trainium2 (8 NeuronCores).

Computes sum of top-k squared singular values of x (= top-k eigenvalues of
the Gram matrix G = x^T x) for x of shape (8192, 4096), k small (k=1).

Algorithm (all device matmuls in bf16 with fp32 PSUM accumulation):
  1. Host: estimate lambda_1 cheaply (block power iteration), scale x by
     1/sqrt(C) and cast to bf16, so the device Gram directly yields A0 = G/C.
  2. Device, sharded across 8 cores (core r owns rows r*512..(r+1)*512 of
     every produced matrix; AllGather of row slices re-assembles plain
     row-major full matrices):
       - g_slice = xcols_r^T @ x          (A0 row-slice)        + AllGather
       - p1 squarings: a_slice = a_slice^T(kxm) @ a_full        + AllGather
         (A_{j+1} = A_j^2; symmetry lets the core's own row-slice, read
          transposed, serve as the lhsT column-slice)
       - block power phase (hand-written, persistent SBUF, `chains`
         independent chains interleaved so one chain's matmuls hide the
         other's AllGather): y_slice = A_p1[rows_r,:] @ y_full, AllGather
       - w = one more application per chain (no AllGather)
       - S1[ci,cj] = Y_ci^T W_cj, S0[ci,cj] = Y_ci^T Y_cj  (partial over the
         core's rows; host sums partials over cores)
  3. Host: generalized Ritz eigenvalues theta_i of (S1, S0) approximate
     lambda_i(A_p1); lambda_i(G) = C * theta_i^(1/2^p1); answer = sum top k.

The 2^-p1 root compresses block-phase and late rounding errors by 2^p1; the
p1 squarings make s block steps act like a degree s*2^p1 polynomial filter.
"""

import numpy as np
import ml_dtypes

N_CORES = 8
M_ROWS = 8192
N_DIM = 4096
P1 = 2
S_STEPS = 9
B_BLOCK = 128
CHAINS = 2

_NC_CACHE: dict = {}


def _est_scale(x_np, iters=15, blk=4):
    """Host block-power-iteration estimate of lambda_1(x^T x).

    Only used to pick the static normalization C; range safety needs C within
    ~±20% of lambda_1, which ~15 block iterations comfortably deliver for any
    PSD spectrum. Returns 1.10 * max Rayleigh quotient (mild overshoot keeps
    the squaring chain's magnitudes shrinking rather than growing).
    """
    rng = np.random.default_rng(0)
    v = rng.standard_normal((x_np.shape[1], blk)).astype(np.float32)
    v /= np.linalg.norm(v, axis=0, keepdims=True)
    for _ in range(iters):
        w = x_np.T @ (x_np @ v)
        v = w / np.linalg.norm(w, axis=0, keepdims=True)
    x64 = x_np.astype(np.float64)
    v64 = v.astype(np.float64)
    v64 /= np.linalg.norm(v64, axis=0, keepdims=True)
    ray = ((x64 @ v64) ** 2).sum(axis=0)
    return 1.10 * float(ray.max())


def _build_nc(m_rows, n_dim, b, p1, s, n_cores, chains, enable_asserts=False):
    import concourse.mybir as mybir
    import concourse.tile as tile
    from concourse import bacc
    import concourse.kernels.tile_matmul as tm
    from contextlib import ExitStack

    orig_comp = tm.composable_matmul_tile_kernel

    def comp_psum2(*a, **kw):
        kw.setdefault("psum_n_bufs", 2)
        return orig_comp(*a, **kw)

    def matmul_tile_kernel(*a, **kw):
        tm.composable_matmul_tile_kernel = comp_psum2
        try:
            return tm.matmul_tile_kernel(*a, **kw)
        finally:
            tm.composable_matmul_tile_kernel = orig_comp

    P = 128
    sl = n_dim // n_cores  # 512 rows per core
    msub = sl // P         # 4
    kpo = n_dim // P       # 32
    bf = mybir.dt.bfloat16
    f32 = mybir.dt.float32
    nc = bacc.Bacc(
        "TRN2",
        target_bir_lowering=False,
        debug=False,
        enable_asserts=enable_asserts,
        num_devices=n_cores,
    )

    xb = nc.dram_tensor("xb", [m_rows, n_dim], bf, kind="ExternalInput")
    xcols = nc.dram_tensor("xcols", [m_rows, sl], bf, kind="ExternalInput")
    omegas = [
        nc.dram_tensor(f"omega{c}", [n_dim, b], bf, kind="ExternalInput")
        for c in range(chains)
    ]
    nb = chains * b
    s1r = nc.dram_tensor("s1r", [nb, nb], f32, kind="ExternalOutput")
    s0r = nc.dram_tensor("s0r", [nb, nb], f32, kind="ExternalOutput")

    g_slice = nc.dram_tensor("g_slice", [sl, n_dim], bf)
    a_full = [
        nc.dram_tensor(f"a_full_{j}", [n_dim, n_dim], bf, addr_space="Shared")
        for j in range(p1)
    ]
    a_slice = [nc.dram_tensor(f"a_slice_{j}", [sl, n_dim], bf) for j in range(p1)]
    y_slice = [
        [nc.dram_tensor(f"y_slice_{c}_{t}", [sl, b], bf) for t in range(s)]
        for c in range(chains)
    ]
    y_full = [
        [
            nc.dram_tensor(f"y_full_{c}_{t}", [n_dim, b], bf, addr_space="Shared")
            for t in range(s)
        ]
        for c in range(chains)
    ]

    rg = [list(range(n_cores))]

    def dve_evict(nc_, psum, sbuf):
        nc_.vector.tensor_copy(out=sbuf, in_=psum)

    def ag(inp, outp):
        nc.gpsimd.collective_compute(
            "AllGather",
            mybir.AluOpType.bypass,
            replica_groups=rg,
            ins=[inp.ap().opt()],
            outs=[outp.ap().opt()],
        )

    with tile.TileContext(nc) as tc:
        # ---- Gram row-slice: G[rows_r, :] = xcols^T @ x ----
        matmul_tile_kernel(
            tc, kxm_ap=xcols.ap(), kxn_ap=xb.ap(), mxn_ap=g_slice.ap(),
            psum_evict_fn=dve_evict,
        )
        prev_s = g_slice
        if p1 > 0:
            ag(g_slice, a_full[0])
            prev_f = a_full[0]
            for j in range(p1):
                matmul_tile_kernel(
                    tc,
                    kxm_ap=prev_s.ap(),
                    kxn_ap=prev_f.ap(),
                    mxn_ap=a_slice[j].ap(),
                    transpose_kxm=True,
                    psum_evict_fn=dve_evict,
                )
                prev_s = a_slice[j]
                if j + 1 < p1:
                    ag(prev_s, a_full[j + 1])
                    prev_f = a_full[j + 1]

        ak = prev_s  # [sl, n_dim] row-slice of A_{p1} (= its column-slice^T)

        # ---- hand-written block power phase ----
        with ExitStack() as ctx:
            cpool = ctx.enter_context(tc.tile_pool(name="blk_const", bufs=1))
            ypool = ctx.enter_context(tc.tile_pool(name="blk_y", bufs=3))
            opool = ctx.enter_context(tc.tile_pool(name="blk_out", bufs=3))
            # PSUM budget: 8 banks total = chains*3 (block) + 2 (S-forms)
            pspool = ctx.enter_context(
                tc.tile_pool(name="blk_psum", bufs=3, space="PSUM")
            )

            # kc[pi, po, f] = A_p1[row f (slice-local), col po*128+pi]
            kc = cpool.tile([P, kpo, sl], bf, tag="kc")
            ak3 = ak.ap().rearrange("f (po pi) -> f po pi", pi=P)
            for kt in range(kpo // 4):
                nc.sync.dma_start_transpose(
                    kc[:, kt * 4 : (kt + 1) * 4, :], ak3[:, kt * 4 : (kt + 1) * 4, :]
                )

            y_cur = []
            for c in range(chains):
                ysb = ypool.tile([P, kpo, b], bf, tag=f"yf{c}")
                nc.sync.dma_start(
                    ysb[:], omegas[c].ap().rearrange("(po pi) b -> pi po b", pi=P)
                )
                y_cur.append(ysb)

            def chain_step(c, dst_sb):
                """dst_sb[pi, mo, :] = (A_p1 @ Y_c)[rows_r] in bf16."""
                for mo in range(msub):
                    ps = pspool.tile([P, b], f32, tag=f"ps{c}")
                    for ko in range(kpo):
                        nc.tensor.matmul(
                            ps[:],
                            kc[:, ko, mo * P : (mo + 1) * P],
                            y_cur[c][:, ko, :],
                            start=(ko == 0),
                            stop=(ko == kpo - 1),
                        )
                    nc.vector.tensor_copy(dst_sb[:, mo, :], ps[:])

            ys_final = [None] * chains
            for t in range(s):
                for c in range(chains):
                    last = t == s - 1
                    if last:
                        out_sb = cpool.tile([P, msub, b], bf, tag=f"ysfin{c}")
                        ys_final[c] = out_sb
                    else:
                        out_sb = opool.tile([P, msub, b], bf, tag=f"yo{c}")
                    chain_step(c, out_sb)
                    nc.sync.dma_start(
                        y_slice[c][t].ap().rearrange("(mo pi) b -> pi mo b", pi=P),
                        out_sb[:],
                    )
                    ag(y_slice[c][t], y_full[c][t])
                    ysb = ypool.tile([P, kpo, b], bf, tag=f"yf{c}")
                    nc.sync.dma_start(
                        ysb[:],
                        y_full[c][t].ap().rearrange("(po pi) b -> pi po b", pi=P),
                    )
                    y_cur[c] = ysb

            # one more application per chain: W_c = A_p1 @ Y_c (slice, no AG)
            w_sb = []
            for c in range(chains):
                wsb = cpool.tile([P, msub, b], bf, tag=f"w{c}")
                chain_step(c, wsb)
                w_sb.append(wsb)

            # ---- S forms: S1[ci,cj] = Y_ci^T W_cj, S0[ci,cj] = Y_ci^T Y_cj ----
            spool = ctx.enter_context(tc.tile_pool(name="s_out", bufs=2))
            pspool2 = ctx.enter_context(
                tc.tile_pool(name="s_psum", bufs=2, space="PSUM")
            )
            for ci in range(chains):
                for cj in range(chains):
                    for which, rhs_sb, out_t in (
                        ("s1", w_sb[cj], s1r),
                        ("s0", ys_final[cj], s0r),
                    ):
                        ps = pspool2.tile([b, b], f32, tag="ps_s")
                        for ko in range(msub):
                            nc.tensor.matmul(
                                ps[:],
                                ys_final[ci][:, ko, :],
                                rhs_sb[:, ko, :],
                                start=(ko == 0),
                                stop=(ko == msub - 1),
                            )
                        osb = spool.tile([b, b], f32, tag="osb")
                        nc.vector.tensor_copy(osb[:], ps[:])
                        nc.sync.dma_start(
                            out_t.ap()[ci * b : (ci + 1) * b, cj * b : (cj + 1) * b],
                            osb[:],
                        )

    nc.compile()
    return nc


def _get_nc(cfg):
    if cfg not in _NC_CACHE:
        _NC_CACHE[cfg] = _build_nc(*cfg)
    return _NC_CACHE[cfg]


def _ritz_topk(S1, S0, k):
    """Top-k generalized eigenvalues of (S1, S0), f64, rank-guarded."""
    S1 = 0.5 * (S1 + S1.T)
    S0 = 0.5 * (S0 + S0.T)
    d = np.sqrt(np.clip(np.diag(S0), 0, None))
    d = np.where(d > 0, d, 1.0)
    dn = 1.0 / d
    S0n = S0 * dn[:, None] * dn[None, :]
    S1n = S1 * dn[:, None] * dn[None, :]
    w0, v0 = np.linalg.eigh(S0n)
    keep = w0 > (w0.max() * 1e-4)
    v = v0[:, keep] / np.sqrt(w0[keep])[None, :]
    m = v.T @ S1n @ v
    m = 0.5 * (m + m.T)
    ev = np.linalg.eigvalsh(m)
    ev = np.clip(ev, 0.0, None)
    return np.sort(ev)[::-1][:k]


def _host_solve(results, k, p1, c_scale):
    S1 = np.zeros_like(results[0]["s1r"], dtype=np.float64)
    S0 = np.zeros_like(results[0]["s0r"], dtype=np.float64)
    for r in results:
        S1 += r["s1r"].astype(np.float64)
        S0 += r["s0r"].astype(np.float64)
    thetas = _ritz_topk(S1, S0, k)
    lams = c_scale * np.power(np.clip(thetas, 1e-300, None), 1.0 / (1 << p1))
    return float(np.sum(lams))


def _make_inputs(x_np, b, n_cores, c_scale, chains):
    n_dim = x_np.shape[1]
    sl = n_dim // n_cores
    bf = ml_dtypes.bfloat16
    xs = (x_np.astype(np.float64) / np.sqrt(c_scale)).astype(np.float32)
    xb = np.ascontiguousarray(xs.astype(bf))
    rng = np.random.default_rng(12345)
    omegas = [
        np.ascontiguousarray(
            rng.standard_normal((n_dim, b)).astype(np.float32).astype(bf)
        )
        for _ in range(chains)
    ]
    in_maps = []
    for r in range(n_cores):
        m = {
            "xb": xb,
            "xcols": np.ascontiguousarray(xb[:, r * sl : (r + 1) * sl]),
        }
        for c in range(chains):
            m[f"omega{c}"] = omegas[c]
        in_maps.append(m)
    return in_maps


def _host_fallback(x_np, k_int):
    """Correct-but-slow host path, used only if the device result is bad."""
    import scipy.linalg

    g = x_np.astype(np.float64).T @ x_np.astype(np.float64)
    n = g.shape[0]
    ev = scipy.linalg.eigh(g, eigvals_only=True, subset_by_index=[n - k_int, n - 1])
    return float(np.sum(ev))


def kernel(x, k):
    from concourse.bass_utils import run_bass_kernel_spmd

    x_np = np.asarray(x, dtype=np.float32)
    k_int = int(np.asarray(k))
    if k_int <= 0:
        return np.asarray(0.0, dtype=np.float32)

    try:
        c_scale = _est_scale(x_np)
        cfg = (M_ROWS, N_DIM, B_BLOCK, P1, S_STEPS, N_CORES, CHAINS)
        nc = _get_nc(cfg)
        in_maps = _make_inputs(x_np, B_BLOCK, N_CORES, c_scale, CHAINS)
        res = run_bass_kernel_spmd(nc, in_maps, core_ids=list(range(N_CORES)))
        val = _host_solve(res.results, k_int, P1, c_scale)
        if not np.isfinite(val) or val <= 0:
            raise FloatingPointError(f"bad device result {val}")
    except Exception:
        val = _host_fallback(x_np, k_int)
    return np.asarray(val, dtype=np.float32)
